# revision 23
# baseline (speedup 1.0000x reference)
"""Trainium2 Bass kernel for sliding-window causal self-attention.

Reference computation (B=1, T=4096, H=8 heads, head_dim=128, DIM=1024):
  qkv = x @ w_qkv.T; q,k = rms_norm -> rope; v = lam0*rms_norm(v) + lam1*ve
  scores = (q k^T) * 0.12 with sliding-window causal mask (0 <= i-j < 512)
  y = softmax(scores) @ v;  out = y @ o_w.T

Sharding over 8 cores: 2 sequence halves (S) x 4 head-pair groups (G).
Core c = 4*s + g handles t in [2048s, 2048(s+1)) for heads {2g, 2g+1}.
Each core reads its x rows plus a 512-row halo of preceding rows (for k/v),
computes its partial output projection over its 2 heads, and the host sums
the 4 partials per half and concatenates the halves. No on-chip collectives.

Attention uses a transposeless [kj, qi] scores layout: q-tiles are processed
in pairs (256 queries, 768-key window, 6 key chunks) so every matmul has a
moving free dim >= 256, which is required for full-rate float32r matmuls.
The sliding-window mask is applied as a -3000 additive bias accumulated into
the scores PSUM by an identity-weighted matmul (so exp() underflows to zero
on masked entries) -- this keeps the mask entirely on the PE and off the DVE.
Softmax runs without max-subtraction (scores are bounded by 0.12*128), the
kj-sum is done on the PE with a ones vector, and the reciprocal is broadcast
across partitions with a Pool-engine partition_broadcast. The output
projection is interleaved into the attention loop (fp16 partials; the host
accumulates in fp32). Elementwise work is spread across DVE / ScalarE /
GpSimd to keep all engines near the PE's occupancy.
"""

import sys

sys.path.insert(0, "/opt/trn_rl_repo")

import numpy as np

import concourse.bass as bass
import concourse.mybir as mybir
import concourse.tile as tile
from concourse import bacc
from concourse.bass_utils import run_bass_kernel_spmd
from concourse.masks import make_identity

# Problem constants
T = 4096
DIM = 1024
H = 8
HD = 128
WINDOW = 512
ATTN_SCALE = 0.12
ROPE_BASE = 1024.0
EPS = 1e-6

# Sharding
S = 2          # sequence halves
G = 4          # head groups (2 heads each)
HPC = H // G   # heads per core = 2
TC = T // S    # own rows per core = 2048
TK = TC + WINDOW  # k/v rows incl. halo = 2560
NQT = TC // 128   # q tiles per head = 16
NKC = TK // 128   # k chunks = 20
NPR = TC // 256   # q pairs per head = 8
PW = 256 + WINDOW  # pair window = 768
NPC = PW // 128    # chunks per pair window = 6
EW = 3 * HPC * HD  # fused qkv width per core = 768

MASK_BIAS = -3000.0  # |scores| <= 128, so -3000 guarantees exp() == 0

F32 = mybir.dt.float32
F32R = mybir.dt.float32r
F16 = mybir.dt.float16

AOP = mybir.AluOpType
AF = mybir.ActivationFunctionType


def build_kernel():
    nc = bacc.Bacc()

    # Per-core DRAM I/O (shapes identical across cores; data differs).
    xT = nc.declare_dram_parameter("xT", [DIM, TK], F32, isOutput=False)
    wqkvT = nc.declare_dram_parameter("wqkvT", [DIM, EW], F32, isOutput=False)
    woT = nc.declare_dram_parameter("woT", [HPC * HD, DIM], F32, isOutput=False)
    ve = nc.declare_dram_parameter("ve", [TK, HPC * HD], F32, isOutput=False)
    # interleaved rope tables ([cos||sin] and [sin||cos] per chunk),
    # pre-permuted host-side to [128, NKC*64] so each partition's slice is
    # one contiguous 5KB DMA descriptor (>=512B avoids the 2x
    # small-descriptor latency penalty).
    cosT = nc.declare_dram_parameter("cosT", [128, NKC * 64], F16, isOutput=False)
    sinT = nc.declare_dram_parameter("sinT", [128, NKC * 64], F16, isOutput=False)
    lam = nc.declare_dram_parameter("lam", [128, 4], F32, isOutput=False)
    padcnt = nc.declare_dram_parameter("padcnt", [512], F32, isOutput=False)
    outT = nc.declare_dram_parameter("outT", [DIM, TC], F16, isOutput=True)

    with tile.TileContext(nc) as tc:
        _trace_body(nc, tc, xT, wqkvT, woT, ve, cosT, sinT, lam, padcnt, outT)

    nc.compile()
    return nc


def _trace_body(nc, tc, xT, wqkvT, woT, ve, cosT, sinT, lam, padcnt, outT):
    import contextlib

    ctx = contextlib.ExitStack()
    with ctx:
        const = ctx.enter_context(tc.tile_pool(name="const", bufs=1))
        persist = ctx.enter_context(tc.tile_pool(name="persist", bufs=1))

        # ---- constants needed by phase A. The k/v weight columns load
        # before the q columns: the first four t-chunks are halo rows that
        # need only k/v, so their projection can start ~6us earlier. ----
        w_sb = const.tile([128, 8, EW], F32R)  # wqkvT as [dpart, dchunk, e]
        wq_r = wqkvT.rearrange("(a p) e -> p a e", p=128).bitcast(F32R)
        # interleaved rope tables: cs1 = [cos||sin], cs2 = [sin||cos] per
        # chunk, so each rope half needs one elementwise multiply
        cs1_sb = const.tile([128, NKC, 2, 32], F16)
        cs2_sb = const.tile([128, NKC, 2, 32], F16)
        lam_sb = const.tile([128, 4], F32)

        identity = const.tile([128, 128], F16)
        idf = const.tile([128, 128], F32)
        make_identity(nc, idf)
        nc.vector.tensor_copy(out=identity, in_=idf)

        eps_sb = const.tile([128, 1], F32)
        nc.vector.memset(eps_sb, EPS)

        # ---- B/C constant tiles (DMAs deferred until after phase A) ----
        wo_sb = const.tile([128, HPC, DIM], F32R)  # woT as [ddpart, head, e]
        pad_r = const.tile([1, 512], F32)  # nonzero only for t < 511

        ones_col = const.tile([128, 1], F16)
        nc.vector.memset(ones_col, 1.0)

        # Band masks (1 valid / 0 invalid) in [kj, ci, qi] orientation for
        # pair-window chunks, fp16 so the DVE mask multiply runs in 4x mode.
        # Chunk c of a pair window is valid iff qi+1 <= 128c + kj <= qi+512.
        # Chunks 2,3 are always fully valid; 0,1 need the lower bound and
        # 4,5 the upper bound.
        maskA = const.tile([128, 2, 256], F16)  # chunks 0,1
        nc.gpsimd.memset(maskA, 1.0)
        nc.gpsimd.affine_select(
            out=maskA, in_=maskA, compare_op=AOP.is_ge, fill=0.0,
            base=-1, channel_multiplier=1, pattern=[[128, 2], [-1, 256]],
        )
        maskB = const.tile([128, 2, 256], F16)  # chunks 4,5
        nc.gpsimd.memset(maskB, 1.0)
        nc.gpsimd.affine_select(
            out=maskB, in_=maskB, compare_op=AOP.is_ge, fill=0.0,
            base=0, channel_multiplier=-1, pattern=[[-128, 2], [1, 256]],
        )

        # ---- persistent activations ----
        # qT/kT: [dd, head, t]; vbf: [t(kj) part, chunk, head, dd];
        # yT: [dd, head, t].
        qT = persist.tile([128, HPC, TC], F16, name="qT")
        kT = persist.tile([128, HPC, TK], F16, name="kT")
        vbf = persist.tile([128, NKC, HPC, HD], F16, name="vbf")
        yT = persist.tile([128, HPC, TC], F32R, name="yT")

        # ================= Phase A: QKV projection + norm/rope =================
        with (
            tc.tile_pool(name="xt_pool", bufs=2) as xt_pool,
            tc.tile_pool(name="ve_pool", bufs=2) as ve_pool,
            tc.tile_pool(name="stageA", bufs=2) as stageA,
            tc.tile_pool(name="st6_pool", bufs=3) as st6_pool,
            tc.tile_pool(name="smallA", bufs=8) as smallA,
            tc.tile_pool(name="proj_psum", bufs=3, space="PSUM") as proj_psum,
            tc.tile_pool(name="tp_psum", bufs=2, space="PSUM") as tp_psum,
        ):
            xT_r = xT.rearrange("(a p) t -> p a t", p=128)  # [128, 8, TK]
            ve_r = ve.rearrange("(a p) d -> p a d", p=128)  # [128, 20, 256]
            TB = 512  # t rows per x block load
            pend = None  # chunk one stage behind: rope combine + transpose

            def flush_pending(pc, pst6, pt12, pt34, ps0):
                # Deferred tail of chunk pc, emitted while chunk pc+1 is in
                # flight so no engine queue ever waits on its own chunk's
                # late producers: rope combines on DVE (fills the gap while
                # ACT runs the next chunk's sqrt), then the q/k transposes
                # on PE (landing after the next projection) and their
                # evacuations on ACT.
                nc.vector.tensor_add(pst6[:, ps0:4, 0:32],
                                     pt12[:, ps0:4, 0, :], pt12[:, ps0:4, 1, :])
                nc.vector.tensor_sub(pst6[:, ps0:4, 64:96],
                                     pt34[:, ps0:4, 1, :], pt34[:, ps0:4, 0, :])
                if pc >= 4:  # q exists only for own rows
                    tq2 = tp_psum.tile([128, 2, 128], F16, name="tq", tag="tp")
                    for h in range(HPC):
                        nc.tensor.transpose(tq2[:, h, :], pst6[:, h, :], identity)
                    nc.scalar.copy(
                        out=qT[:, :, (pc - 4) * 128:(pc - 3) * 128], in_=tq2)
                tk2 = tp_psum.tile([128, 2, 128], F16, name="tk", tag="tp")
                for h in range(HPC):
                    nc.tensor.transpose(tk2[:, h, :], pst6[:, 2 + h, :], identity)
                nc.scalar.copy(
                    out=kT[:, :, pc * 128:(pc + 1) * 128], in_=tk2)

            for tb in range(TK // TB):
                xt = xt_pool.tile([128, 8, TB], F32R)
                # split into two DMAs so compute can start on the first half;
                # on the first block the k/v weight columns load between the
                # x halves (halo chunks 0-3 need only k/v), the q columns and
                # the small tables after
                xsrc = xT_r[:, :, tb * TB:(tb + 1) * TB].bitcast(F32R)
                if tb == 0:
                    # k/v weights first, then x in single-chunk slices so the
                    # first projection completes as early as possible
                    nc.sync.dma_start(out=w_sb[:, 0:4, 256:EW],
                                      in_=wq_r[:, 0:4, 256:EW])
                    nc.sync.dma_start(out=w_sb[:, 4:8, 256:EW],
                                      in_=wq_r[:, 4:8, 256:EW])
                    for cc in range(4):
                        nc.sync.dma_start(
                            out=xt[:, :, cc * 128:(cc + 1) * 128],
                            in_=xsrc[:, :, cc * 128:(cc + 1) * 128])
                else:
                    nc.sync.dma_start(out=xt[:, 0:4, :], in_=xsrc[:, 0:4, :])
                    nc.sync.dma_start(out=xt[:, 4:8, :], in_=xsrc[:, 4:8, :])
                vet = ve_pool.tile([128, 4, HPC * HD], F32)
                if tb == 0:
                    # q weight columns deferred behind the halo chunks
                    nc.sync.dma_start(out=w_sb[:, :, 0:256],
                                      in_=wq_r[:, :, 0:256])
                nc.sync.dma_start(out=vet, in_=ve_r[:, tb * 4:(tb + 1) * 4, :])
                if tb == 0:
                    nc.sync.dma_start(
                        out=cs1_sb,
                        in_=cosT.rearrange("p (a h f) -> p a h f", h=2, f=32))
                    nc.sync.dma_start(
                        out=cs2_sb,
                        in_=sinT.rearrange("p (a h f) -> p a h f", h=2, f=32))
                    nc.sync.dma_start(out=lam_sb, in_=lam[:])
                if tb == 3:
                    # phase B/C constants: DMA is idle by now, and loading
                    # here avoids a stall at the phase A->B boundary
                    nc.sync.dma_start(
                        out=wo_sb,
                        in_=woT.rearrange("(a p) e -> p a e", p=128).bitcast(F32R))
                    nc.sync.dma_start(
                        out=pad_r, in_=padcnt.rearrange("(a t) -> a t", a=1))

                for tsub in range(TB // 128):
                    c = tb * (TB // 128) + tsub  # t-chunk index, 0..19
                    psum = proj_psum.tile([128, EW], F32)
                    for dch in range(8):
                        lhsT = xt[:, dch, tsub * 128:(tsub + 1) * 128]
                        if c >= 4:
                            nc.tensor.matmul(
                                psum[:, 0:512], lhsT, w_sb[:, dch, 0:512],
                                start=(dch == 0), stop=(dch == 7),
                            )
                        else:  # halo rows need only k,v
                            nc.tensor.matmul(
                                psum[:, 256:512], lhsT, w_sb[:, dch, 256:512],
                                start=(dch == 0), stop=(dch == 7),
                            )
                        nc.tensor.matmul(
                            psum[:, 512:EW], lhsT, w_sb[:, dch, 512:EW],
                            start=(dch == 0), stop=(dch == 7),
                        )
                    # psum segments: q0 q1 k0 k1 v0 v1, each [128, 128]
                    psum6 = psum.rearrange("p (s d) -> p s d", s=6)

                    # RMS-norm scales (halo chunks skip the q segments):
                    # one big Square on ACT, per-segment row sums on DVE
                    s0 = 0 if c >= 4 else 2
                    sq = stageA.tile([128, 6, HD], F32)
                    nc.scalar.activation(sq[:, s0:6, :], psum6[:, s0:6, :],
                                         AF.Square)
                    ssum = smallA.tile([128, 6], F32)
                    nc.vector.tensor_reduce(
                        out=ssum[:, s0:6], in_=sq[:, s0:6, :],
                        axis=mybir.AxisListType.X, op=AOP.add)

                    if pend is not None:
                        flush_pending(*pend)
                        pend = None
                    # rms for q,k (eps bias) and v (lam0 folded via scale/bias)
                    rms = smallA.tile([128, 6], F32)
                    nc.scalar.activation(rms[:, s0:4], ssum[:, s0:4], AF.Sqrt,
                                         bias=eps_sb, scale=1.0 / HD)
                    nc.scalar.activation(rms[:, 4:6], ssum[:, 4:6], AF.Sqrt,
                                         bias=lam_sb[:, 3:4],
                                         scale=lam_sb[:, 2:3])
                    rs = smallA.tile([128, 6], F32)
                    nc.vector.reciprocal(rs[:, s0:6], rms[:, s0:6])

                    # normalize segments in one DVE op -> staging (f32r)
                    st6 = st6_pool.tile([128, 6, HD], F16)
                    nc.vector.tensor_tensor(
                        out=st6[:, s0:6, :], in0=psum6[:, s0:6, :],
                        in1=rs[:, s0:6, None].to_broadcast([128, 6 - s0, HD]),
                        op=AOP.mult,
                    )
                    st6f = st6

                    # v = lam1 * ve + v_normed; both steps on the Pool engine
                    # (all operands SBUF), writing both heads in one op.
                    vel = stageA.tile([128, 2, HD], F32, name="vel")
                    nc.gpsimd.tensor_tensor(
                        out=vel, in0=vet[:, tsub, :].rearrange("p (h d) -> p h d", h=2),
                        in1=lam_sb[:, 1:2, None].to_broadcast([128, 2, HD]),
                        op=AOP.mult,
                    )

                    # rope on q,k (dims 0:32 rotate with dims 64:96): the
                    # rotating halves are multiplied by the interleaved
                    # [cos||sin] / [sin||cos] tables in two Pool ops, then
                    # combined with one DVE add and one DVE subtract
                    nseg = 4 - s0
                    xr = st6f[:, s0:4, 0:96].rearrange(
                        "p s (h f) -> p s h f", f=32)[:, :, 0:3:2, :]
                    t12 = stageA.tile([128, 4, 2, 32], F16)
                    t34 = stageA.tile([128, 4, 2, 32], F16)
                    nc.gpsimd.tensor_tensor(
                        out=t12[:, s0:4], in0=xr,
                        in1=cs1_sb[:, c:c + 1].to_broadcast([128, nseg, 2, 32]),
                        op=AOP.mult)
                    nc.gpsimd.tensor_tensor(
                        out=t34[:, s0:4], in0=xr,
                        in1=cs2_sb[:, c:c + 1].to_broadcast([128, nseg, 2, 32]),
                        op=AOP.mult)
                    nc.gpsimd.tensor_tensor(
                        out=vbf[:, c, :, :], in0=vel, in1=st6f[:, 4:6, :],
                        op=AOP.add,
                    )
                    pend = (c, st6, t12, t34, s0)

            flush_pending(*pend)

        # ====== Phase B+C: banded attention with interleaved out-projection ===
        with (
            tc.tile_pool(name="pm_pool", bufs=3) as pm_pool,
            tc.tile_pool(name="smallB", bufs=8) as smallB,
            tc.tile_pool(name="o_out", bufs=4) as o_out,
            tc.tile_pool(name="sc_psum", bufs=3, space="PSUM") as sc_psum,
            tc.tile_pool(name="sum_psum", bufs=1, space="PSUM") as sum_psum,
            tc.tile_pool(name="y_psum", bufs=2, space="PSUM") as y_psum,
            tc.tile_pool(name="o_psum", bufs=2, space="PSUM") as o_psum,
        ):
            def oproj_window(tw):
                # out[:, 512tw:512tw+512] = sum_h woT_h^T @ yT_h window
                for ec in range(8):
                    ops = o_psum.tile([128, 512], F32, name="ops")
                    for h in range(HPC):
                        nc.tensor.matmul(
                            ops,
                            wo_sb[:, h, ec * 128:(ec + 1) * 128],
                            yT[:, h, tw * 512:(tw + 1) * 512],
                            start=(h == 0), stop=(h == HPC - 1),
                            skip_group_check=True,
                        )
                    ot = o_out.tile([128, 512], F16, name="ot")
                    if ec % 2 == 0:
                        nc.scalar.copy(out=ot, in_=ops)
                    else:
                        nc.vector.tensor_copy(out=ot, in_=ops)
                    nc.sync.dma_start(
                        out=outT[ec * 128:(ec + 1) * 128,
                                 tw * 512:(tw + 1) * 512],
                        in_=ot,
                    )

            # Cross-step software pipeline: step n+1's score matmuls (and
            # their exps) are issued before step n's twelve accumulation
            # matmuls, so the exp chain of the next step runs on ACT while
            # the PE drains the current accumulation.
            def issue_step(pr, h):
                qs = qT[:, h, pr * 256:(pr + 1) * 256]
                pm = pm_pool.tile([128, NPC, 256], F16, name="pm")
                for wp in (0, 2, 1):  # chunk pairs, masked pairs first
                    sc = sc_psum.tile([128, 2, 256], F32, name="sc", tag="sc")
                    for j in range(2):
                        wc = 2 * wp + j
                        nc.tensor.matmul(
                            sc[:, j, :],
                            kT[:, h, (2 * pr + wc) * 128:(2 * pr + wc + 1) * 128],
                            qs, start=True, stop=True,
                            skip_group_check=True,
                        )
                    nc.scalar.activation(pm[:, 2 * wp:2 * wp + 2, :], sc,
                                         AF.Exp, scale=ATTN_SCALE)
                    if wp == 0:
                        nc.vector.tensor_tensor(
                            out=pm[:, 0:2, :], in0=pm[:, 0:2, :],
                            in1=maskA, op=AOP.mult)
                    elif wp == 2:
                        nc.vector.tensor_tensor(
                            out=pm[:, 4:6, :], in0=pm[:, 4:6, :],
                            in1=maskB, op=AOP.mult)
                return pm

            def consume_step(pr, h, pm):
                sums = sum_psum.tile([1, 256], F32, name="sums")
                yps = y_psum.tile([128, 256], F32, name="yps")
                for i, wp in enumerate((0, 2, 1)):
                    for j in range(2):
                        wc = 2 * wp + j
                        nc.tensor.matmul(
                            sums, ones_col, pm[:, wc, :],
                            start=(i == 0 and j == 0),
                            stop=(i == 2 and j == 1),
                            skip_group_check=True,
                        )
                        nc.tensor.matmul(
                            yps, vbf[:, 2 * pr + wc, h, :], pm[:, wc, :],
                            start=(i == 0 and j == 0),
                            stop=(i == 2 and j == 1),
                            skip_group_check=True,
                        )
                with tc.high_priority(offset=40):
                    recip = smallB.tile([1, 256], F32)
                    if pr < 2:
                        sums2 = smallB.tile([1, 256], F32)
                        nc.vector.tensor_sub(sums2, sums,
                                             pad_r[:, pr * 256:(pr + 1) * 256])
                        nc.vector.reciprocal(recip, sums2)
                    else:
                        nc.vector.reciprocal(recip, sums)
                    # broadcast 1/sum across partitions on the Pool engine
                    bc_sb = smallB.tile([128, 256], F32, name="bc_sb")
                    nc.gpsimd.partition_broadcast(bc_sb, recip)
                # evacuate with the 1/sum normalization fused (cast f32r)
                nc.vector.tensor_tensor(
                    out=yT[:, h, pr * 256:(pr + 1) * 256],
                    in0=yps, in1=bc_sb, op=AOP.mult)
                if h == HPC - 1 and pr % 2 == 1:
                    oproj_window(pr // 2)

            steps = [(pr, h) for pr in range(NPR) for h in range(HPC)]
            pms = [issue_step(*steps[0])]
            for i, (pr, h) in enumerate(steps):
                if i + 1 < len(steps):
                    pms.append(issue_step(*steps[i + 1]))
                consume_step(pr, h, pms[i])
                pms[i] = None


_NC_CACHE = None


def _get_nc():
    global _NC_CACHE
    if _NC_CACHE is None:
        _NC_CACHE = build_kernel()
    return _NC_CACHE


def _rope_tables(positions):
    keep = HD // 4
    active = (1.0 / ROPE_BASE) ** np.linspace(0.0, 1.0, keep, dtype=np.float32)
    theta = positions[:, None].astype(np.float32) * active[None, :]  # [n, 32]
    return np.cos(theta).astype(np.float32), np.sin(theta).astype(np.float32)


def make_in_maps(x, ve, lambdas, qkvo_w):
    """Build the 8 per-core input maps from full inputs (host-side sharding)."""
    x2 = x.reshape(T, DIM)
    ve2 = ve.reshape(T, DIM)
    qw, kw, vw, ow = qkvo_w[0], qkvo_w[1], qkvo_w[2], qkvo_w[3]

    in_maps = []
    for c in range(8):
        s, g = divmod(c, G)
        h0, h1 = HPC * g, HPC * g + 1
        lo = TC * s - WINDOW  # first k/v row (may be negative -> zero pad)
        hi = TC * s + TC

        # xT slice with zero pad
        xs = np.zeros((TK, DIM), np.float32)
        src_lo = max(lo, 0)
        xs[src_lo - lo:, :] = x2[src_lo:hi, :]
        xTc = np.ascontiguousarray(xs.T)

        # fused qkv weight, transposed: cols = q0 q1 k0 k1 v0 v1
        wcols = []
        for wmat in (qw, kw, vw):
            for h in (h0, h1):
                wcols.append(wmat[h * HD:(h + 1) * HD, :].T)
        wqkvT = np.ascontiguousarray(np.concatenate(wcols, axis=1))

        woT = np.ascontiguousarray(ow[:, h0 * HD:(h1 + 1) * HD].T)

        ves = np.zeros((TK, HPC * HD), np.float32)
        ves[src_lo - lo:, :] = ve2[src_lo:hi, h0 * HD:(h1 + 1) * HD]

        pos = np.clip(np.arange(lo, hi), 0, None)
        cosF, sinF = _rope_tables(pos)  # [TK, 32]
        # interleaved rope tables, permuted so partition p holds its chunks
        # contiguously: cs1 = [cos||sin] and cs2 = [sin||cos] per chunk,
        # each [128, NKC*64]
        cs = np.stack([cosF, sinF], axis=1).reshape(NKC, 128, 64)
        sc = np.stack([sinF, cosF], axis=1).reshape(NKC, 128, 64)
        cosP = np.ascontiguousarray(
            cs.transpose(1, 0, 2).reshape(128, NKC * 64)).astype(np.float16)
        sinP = np.ascontiguousarray(
            sc.transpose(1, 0, 2).reshape(128, NKC * 64)).astype(np.float16)

        l0, l1 = float(lambdas[0]), float(lambdas[1])
        lam_row = np.array([l0, l1, 1.0 / (HD * l0 * l0), EPS / (l0 * l0)],
                           np.float32)
        lam = np.tile(lam_row.reshape(1, 4), (128, 1)).astype(np.float32)

        pc = np.zeros(512, np.float32)
        if s == 0:
            i = np.arange(512)
            pc = np.maximum(0.0, WINDOW - 1.0 - i).astype(np.float32)

        in_maps.append({
            "xT": xTc, "wqkvT": wqkvT, "woT": woT, "ve": ves,
            "cosT": cosP, "sinT": sinP, "lam": lam, "padcnt": pc,
        })
    return in_maps


def kernel(x, ve, lambdas, qkvo_w, window):
    assert int(window) == WINDOW
    x = np.asarray(x, np.float32)
    ve = np.asarray(ve, np.float32)
    lambdas = np.asarray(lambdas, np.float32)
    qkvo_w = np.asarray(qkvo_w, np.float32)

    nc = _get_nc()
    in_maps = make_in_maps(x, ve, lambdas, qkvo_w)
    res = run_bass_kernel_spmd(nc, in_maps, core_ids=list(range(8)))

    outT_full = np.zeros((DIM, T), np.float32)
    for c in range(8):
        s = c // G
        outT_full[:, TC * s:TC * (s + 1)] += res.results[c]["outT"].astype(np.float32)
    return np.ascontiguousarray(outT_full.T).reshape(1, T, DIM)


if __name__ == "__main__":
    nc = _get_nc()
    print("kernel built ok")
